# revision 61
# baseline (speedup 1.0000x reference)
"""NeuralCDE RK4 solver as a Bass/Tile kernel on 8 Trainium2 cores.

Data-parallel over batch: B=1024 -> 128 rows per core. The 127-step RK4
scan is fully unrolled. Two key restructurings vs a naive lowering:

1. The MLP output f is produced in a TRANSPOSED layout fT[(h',c), (j,b)]
   via 4 column-chunked mm2 matmuls, so the einsum
   k[b,h] = sum_c f[b,h,c]*g[b,c] fuses into the NEXT stage's mm1 using
   replicated weights W1Rep[(h',c),m] = alpha*W1[16j+h',m]. This removes
   the tensor_reduce, the PE transpose, and the alpha-scale from the
   per-stage critical chain:
       mul (DVE) -> 4x mm1acc (PE) -> relu (DVE) -> 4x mm2 (PE) -> tanh (ACT)
2. The 128 batch rows per core are split into two 64-row halves whose
   serial chains interleave on the engines (software pipelining), roughly
   halving the per-stage latency.

The z-update k-sums come from small side matmuls (S selection matrices,
RK4 weights folded in) accumulating into accP[64,128] PSUM off the
critical path; z' = z + accP via one DVE op, fp16 copy via ACT.
g (the dX/dt factors, partition-replicated) streams from DRAM per step.
"""

import numpy as np
import ml_dtypes

import concourse.bacc as bacc
import concourse.bass as bass
import concourse.mybir as mybir
from concourse.tile import TileContext
from concourse.bass_utils import run_bass_kernel_spmd

F32 = mybir.dt.float32
FP16 = mybir.dt.float16
B = 1024
L = 128
C_IN = 8
HID = 64
MLP_H = 128
NSTEP = L - 1  # 127
NCORES = 8
BL = B // NCORES  # 128 batch rows per core
NF = HID * C_IN  # 512
NH = 2  # batch halves per core (software pipeline)
HB = BL // NH  # 64

_CACHE: dict = {}


def _build(nstep: int, with_b2: bool):
    import time as _time
    import sys

    t0 = _time.time()
    nc = bacc.Bacc()
    grep_in = nc.dram_tensor("grep", [BL, nstep * 3 * BL], FP16, kind="ExternalInput")
    b1_in = nc.dram_tensor("bias1", [MLP_H, nstep * 3], F32, kind="ExternalInput")
    w1z_in = nc.dram_tensor("w1z", [HID, MLP_H], FP16, kind="ExternalInput")
    w1a_in = nc.dram_tensor("w1a", [MLP_H, 4 * MLP_H], FP16, kind="ExternalInput")
    w1b_in = nc.dram_tensor("w1b", [MLP_H, 4 * MLP_H], FP16, kind="ExternalInput")
    w1c_in = nc.dram_tensor("w1c", [MLP_H, 4 * MLP_H], FP16, kind="ExternalInput")
    s6_in = nc.dram_tensor("s6", [MLP_H, 4 * HID], FP16, kind="ExternalInput")
    w2_in = nc.dram_tensor("w2", [MLP_H, NF], FP16, kind="ExternalInput")
    z0t_in = nc.dram_tensor("z0t", [HID, BL], F32, kind="ExternalInput")
    z0h_in = nc.dram_tensor("z0h", [HID, BL], FP16, kind="ExternalInput")
    if with_b2:
        b2c_in = nc.dram_tensor("b2c", [4, MLP_H], FP16, kind="ExternalInput")
        jsel_in = nc.dram_tensor("jsel", [4, 4 * HB], FP16, kind="ExternalInput")
    zs_out = nc.dram_tensor("zs", [HID, (nstep + 1) * BL], F32, kind="ExternalOutput")

    CLS = (0, 1, 1, 2)
    Tanh = mybir.ActivationFunctionType.Tanh
    Copy = mybir.ActivationFunctionType.Copy
    HNF = 4 * HB  # 256, per-half f width

    with TileContext(nc) as tc:
        with (
            tc.tile_pool(name="const", bufs=1) as cp,
            tc.tile_pool(name="zst", bufs=1) as zp,
            tc.tile_pool(name="g", bufs=6) as gp,
            tc.tile_pool(name="hs", bufs=3 * NH) as hp,
            tc.tile_pool(name="fs", bufs=2 * NH) as fp,
            tc.tile_pool(name="us", bufs=3 * NH) as up,
            tc.tile_pool(name="sc", bufs=2) as scp,
            tc.tile_pool(name="zh", bufs=3) as zhp,
            tc.tile_pool(name="ph", bufs=2, space="PSUM") as ph,
            tc.tile_pool(name="pf", bufs=1, space="PSUM") as pf,
            tc.tile_pool(name="pa", bufs=1, space="PSUM") as pa,
            tc.tile_pool(name="pd", bufs=1, space="PSUM") as pd,
        ):
            b1S = cp.tile([MLP_H, nstep * 3], F32)
            w1zS = cp.tile([HID, MLP_H], FP16)
            w1aS = cp.tile([MLP_H, 4 * MLP_H], FP16)
            w1bS = cp.tile([MLP_H, 4 * MLP_H], FP16)
            w1cS = cp.tile([MLP_H, 4 * MLP_H], FP16)
            s6S = cp.tile([MLP_H, 4 * HID], FP16)
            w2S = cp.tile([MLP_H, NF], FP16)
            zall = zp.tile([HID, (nstep + 1) * BL], F32)

            nc.sync.dma_start(out=b1S[:], in_=b1_in[:])
            nc.sync.dma_start(out=w1zS[:], in_=w1z_in[:])
            nc.sync.dma_start(out=w1aS[:], in_=w1a_in[:])
            nc.sync.dma_start(out=w1bS[:], in_=w1b_in[:])
            nc.sync.dma_start(out=w1cS[:], in_=w1c_in[:])
            nc.sync.dma_start(out=s6S[:], in_=s6_in[:])
            nc.sync.dma_start(out=w2S[:], in_=w2_in[:])
            if with_b2:
                b2cS = cp.tile([4, MLP_H], FP16)
                jselS = cp.tile([4, 4 * HB], FP16)
                nc.sync.dma_start(out=b2cS[:], in_=b2c_in[:])
                nc.sync.dma_start(out=jselS[:], in_=jsel_in[:])
            nc.sync.dma_start(out=zall[:, 0:BL], in_=z0t_in[:])
            nc.sync.dma_start(out=zs_out[:, 0:BL], in_=z0t_in[:])
            zh_cur = zhp.tile([HID, BL], FP16, name="zh0")
            nc.sync.dma_start(out=zh_cur[:], in_=z0h_in[:])

            # pre-create h_ps tiles for step 0 stage 0 and run the bases
            # (no kh accumulate for the very first stage)
            h_ps_next = []
            for h in range(NH):
                bs = slice(h * HB, (h + 1) * HB)
                hp_t = ph.tile([MLP_H, HB], F32, tag=f"hps{h}")
                h_ps_next.append(hp_t)
                nc.tensor.matmul(
                    hp_t[:], lhsT=w1zS[:], rhs=zh_cur[:, bs], start=True, stop=True
                )

            u_stage = [[None] * NH for _ in range(4)]
            sc = [None] * NH

            # scratch bank for PE gap-filler matmuls: dependency-free work
            # that keeps the PE clock ramped through the tanh/mul stalls
            dummyP = pd.tile([MLP_H, NF], F32, name="dummy")

            def emit_fill(fs_t):
                # gap-bridge: depends on the tanh output, so it starts
                # right when the PE runs dry during the mul window and
                # keeps the clock ramped until the mm1-accs arrive.
                # Split into small matmuls so a ready real op is delayed
                # by at most ~one N=128 matmul.
                for i in range(3):
                    off = (i & 1) * MLP_H
                    nc.tensor.matmul(
                        dummyP[:, 0:MLP_H],
                        lhsT=w2S[:, 0:MLP_H],
                        rhs=fs_t[:, off : off + MLP_H],
                        start=True,
                        stop=True,
                    )

            def emit_sred(accP_, scsh_, u3_):
                # delta-z = (1/6) S^T (sc + u3) into accP, off the
                # critical path. The sc part reads both halves of the
                # shared sc tile via a 2D AP (N=128); the first matmul
                # covers the full accP region so start=True is safe.
                sc4 = scsh_[:].rearrange("p (h j b) -> p j h b", h=NH, j=4)
                for j in range(4):
                    nc.tensor.matmul(
                        accP_[:],
                        lhsT=s6S[:, j * HID : (j + 1) * HID],
                        rhs=sc4[:, j],
                        start=(j == 0),
                        stop=False,
                        skip_group_check=(j > 0),
                    )
                for h in range(NH):
                    for j in range(4):
                        nc.tensor.matmul(
                            accP_[:, h * HB : (h + 1) * HB],
                            lhsT=s6S[:, j * HID : (j + 1) * HID],
                            rhs=u3_[h][:, j * HB : (j + 1) * HB],
                            start=False,
                            stop=(h == NH - 1 and j == 3),
                            skip_group_check=True,
                        )

            pending = None  # previous step's (accP, sc, u3, step index)
            fy_pending = None  # fS of the trailing half, for gap-bridging
            for step in range(nstep):
                gslot = gp.tile([BL, 3 * BL], FP16, tag="g")
                nc.sync.dma_start(
                    out=gslot[:], in_=grep_in[:, step * 3 * BL : (step + 1) * 3 * BL]
                )
                accP = pa.tile([HID, BL], F32, tag="acc")
                for s in range(4):
                    col = step * 3 + CLS[s]
                    h_ps = h_ps_next
                    hS = []
                    f_ps = []
                    fS = []
                    # PE: chain mm1-accs (bases were hoisted into the
                    # previous stage's idle window; stage 0's h was fully
                    # accumulated from sc/u3 during the previous step)
                    for h in range(NH):
                        if s >= 1:
                            if h == 1 and fy_pending is not None:
                                emit_fill(fy_pending)
                                fy_pending = None
                            # u from cls-1 stages carries a folded 2x, so
                            # stage 3 (alpha=0.5 on 2*k2) uses the 0.25
                            # variant; stages 2 and 4 use the 0.5 variant
                            wrep = w1bS if s == 2 else w1aS
                            for j in range(4):
                                nc.tensor.matmul(
                                    h_ps[h][:],
                                    lhsT=wrep[:, j * MLP_H : (j + 1) * MLP_H],
                                    rhs=u_stage[s - 1][h][:, j * HB : (j + 1) * HB],
                                    start=False,
                                    stop=(j == 3),
                                )
                    # DVE: relus
                    for h in range(NH):
                        hs_t = hp.tile([MLP_H, HB], FP16, tag=f"hs{h}")
                        hS.append(hs_t)
                        nc.vector.tensor_scalar(
                            hs_t[:],
                            h_ps[h][:],
                            b1S[:, col : col + 1],
                            0.0,
                            op0=mybir.AluOpType.add,
                            op1=mybir.AluOpType.max,
                        )
                    # DVE: RK4 u-presums for the PREVIOUS stage, emitted
                    # between this stage's relus and muls so they fill
                    # the DVE tanh-window without delaying either. The
                    # final sum lands in a shared tile (halves in column
                    # ranges) so the S-red can read it with one 2D AP.
                    if s == 2:
                        for h in range(NH):
                            sc_t = scp.tile([MLP_H, HNF], FP16, tag=f"sci{h}")
                            nc.vector.tensor_tensor(
                                out=sc_t[:],
                                in0=u_stage[1][h][:],
                                in1=u_stage[0][h][:],
                                op=mybir.AluOpType.add,
                            )
                            sc[h] = sc_t
                    elif s == 3:
                        scsh = scp.tile([MLP_H, NH * HNF], FP16, tag="scsh")
                        for h in range(NH):
                            nc.vector.tensor_tensor(
                                out=scsh[:, h * HNF : (h + 1) * HNF],
                                in0=u_stage[2][h][:],
                                in1=sc[h][:],
                                op=mybir.AluOpType.add,
                            )
                    # PE: mm2 chunks
                    for h in range(NH):
                        fp_t = pf.tile([MLP_H, HNF], F32, tag=f"fps{h}")
                        f_ps.append(fp_t)
                        if with_b2:
                            nc.tensor.matmul(
                                fp_t[:],
                                lhsT=b2cS[:],
                                rhs=jselS[:],
                                start=True,
                                stop=False,
                            )
                        for j in range(4):
                            nc.tensor.matmul(
                                fp_t[:, j * HB : (j + 1) * HB],
                                lhsT=w2S[:, j * BL : (j + 1) * BL],
                                rhs=hS[h][:],
                                start=not with_b2,
                                stop=True,
                                skip_group_check=with_b2,
                            )
                    # Previous step's z-update: S-red matmuls (PE, filling
                    # the stage-0 tanh window), then the f32 z update and
                    # the fp16 z copy on DVE (queued after the relus, so
                    # they cannot delay the chain). Emission order is
                    # writer-before-reader throughout.
                    if s == 0 and pending is not None:
                        pacc, psc, pu3, pstep = pending
                        emit_sred(pacc, psc, pu3)
                        nxt_sl = zall[:, (pstep + 1) * BL : (pstep + 2) * BL]
                        nc.vector.scalar_tensor_tensor(
                            out=nxt_sl,
                            in0=pacc[:],
                            scalar=1.0,
                            in1=zall[:, pstep * BL : (pstep + 1) * BL],
                            op0=mybir.AluOpType.mult,
                            op1=mybir.AluOpType.add,
                        )
                        zh_cur = zhp.tile([HID, BL], FP16, tag="zh")
                        nc.vector.tensor_scalar_mul(zh_cur[:], nxt_sl, 1.0)
                        nc.gpsimd.dma_start(
                            out=zs_out[:, (pstep + 1) * BL : (pstep + 2) * BL],
                            in_=nxt_sl,
                        )
                        pending = None
                    # PE: hoist the next stage's bases into this stage's
                    # tanh/mul window (rhs is always the current step's z).
                    # At s==3 also accumulate the next step's stage-0 h
                    # contribution from sc = u0+u1+u2 (weights (1/6)W1Rep),
                    # so the step boundary has no extra chain ops.
                    if not (s == 3 and step == nstep - 1):
                        h_ps_next = []
                        for h in range(NH):
                            bs = slice(h * HB, (h + 1) * HB)
                            hp_t = ph.tile([MLP_H, HB], F32, tag=f"hps{h}")
                            h_ps_next.append(hp_t)
                            nc.tensor.matmul(
                                hp_t[:],
                                lhsT=w1zS[:],
                                rhs=zh_cur[:, bs],
                                start=True,
                                stop=False,
                            )
                        if s == 3:
                            for h in range(NH):
                                for j in range(4):
                                    nc.tensor.matmul(
                                        h_ps_next[h][:],
                                        lhsT=w1cS[:, j * MLP_H : (j + 1) * MLP_H],
                                        rhs=scsh[
                                            :, h * HNF + j * HB : h * HNF + (j + 1) * HB
                                        ],
                                        start=False,
                                        stop=False,
                                    )
                    # ACT: tanh
                    for h in range(NH):
                        fs_t = fp.tile([MLP_H, HNF], FP16, tag=f"fs{h}")
                        fS.append(fs_t)
                        nc.scalar.activation(fs_t[:], f_ps[h][:], Tanh)
                    # DVE: mul by g
                    for h in range(NH):
                        u = up.tile([MLP_H, HNF], FP16, tag=f"u{h}")
                        u3 = u[:].rearrange("p (j b) -> p j b", j=4)
                        f3 = fS[h][:].rearrange("p (j b) -> p j b", j=4)
                        gv = (
                            gslot[:, CLS[s] * BL + h * HB : CLS[s] * BL + (h + 1) * HB]
                            .unsqueeze(1)
                            .broadcast_to((BL, 4, HB))
                        )
                        nc.vector.tensor_tensor(
                            out=u3, in0=f3, in1=gv, op=mybir.AluOpType.mult
                        )
                        u_stage[s][h] = u
                    # PE gap-bridge for this stage's X half (queued after
                    # the hoisted bases, fires at tanh-X completion). At
                    # stage 0 the S-red matmuls already fill this window.
                    if s > 0:
                        emit_fill(fS[0])
                    fy_pending = fS[1]
                # PE: chain — stage-0 (next step) u3 contribution right
                # after the stage-3 muls; closes the h_ps group
                if step < nstep - 1:
                    for h in range(NH):
                        if h == 1 and fy_pending is not None:
                            emit_fill(fy_pending)
                            fy_pending = None
                        for j in range(4):
                            nc.tensor.matmul(
                                h_ps_next[h][:],
                                lhsT=w1cS[:, j * MLP_H : (j + 1) * MLP_H],
                                rhs=u_stage[3][h][:, j * HB : (j + 1) * HB],
                                start=False,
                                stop=(j == 3),
                            )
                pending = (accP, scsh, list(u_stage[3]), step)
            # final step's z-update (no next stage-0 block to host it)
            pacc, psc, pu3, pstep = pending
            emit_sred(pacc, psc, pu3)
            nxt_sl = zall[:, (pstep + 1) * BL : (pstep + 2) * BL]
            nc.vector.scalar_tensor_tensor(
                out=nxt_sl,
                in0=pacc[:],
                scalar=1.0,
                in1=zall[:, pstep * BL : (pstep + 1) * BL],
                op0=mybir.AluOpType.mult,
                op1=mybir.AluOpType.add,
            )
            nc.gpsimd.dma_start(
                out=zs_out[:, (pstep + 1) * BL : (pstep + 2) * BL], in_=nxt_sl
            )

    print(f"[kernel] tile trace+schedule: {_time.time()-t0:.1f}s", file=sys.stderr)
    t1 = _time.time()
    nc.finalize()
    print(f"[kernel] finalize: {_time.time()-t1:.1f}s", file=sys.stderr)
    return nc


def _get_nc(nstep: int, with_b2: bool):
    key = (nstep, with_b2)
    if key not in _CACHE:
        _CACHE[key] = _build(nstep, with_b2)
    return _CACHE[key]


def _host_prep(coeffs, Wi1, bi1, Wi2, bi2, W1, b1, W2, b2, nstep: int):
    coeffs = np.asarray(coeffs, dtype=np.float32)
    a = coeffs[:, :, 0:8]
    b = coeffs[:, :, 8:16]
    c = coeffs[:, :, 16:24]
    d = coeffs[:, :, 24:32]

    X0 = a[:, 0]
    z0 = np.tanh(
        np.maximum(X0 @ Wi1 + bi1, 0.0).astype(np.float32) @ Wi2 + bi2
    ).astype(np.float32)

    # g[b, i, cls, c] = dX/dt at stage times (cls 0: t=i, 1: t=i+.5, 2: t=i+1)
    g = np.empty((B, nstep, 3, C_IN), dtype=np.float32)
    g[:, :, 0] = b[:, :nstep]
    # cls 1 (t = i + 0.5) carries a folded 2x RK4 weight
    g[:, :, 1] = 2.0 * (b[:, :nstep] + c[:, :nstep] + 0.75 * d[:, :nstep])
    for i in range(nstep):
        if i + 1 < L - 1:
            g[:, i, 2] = b[:, i + 1]
        else:
            g[:, i, 2] = b[:, i] + 2.0 * c[:, i] + 3.0 * d[:, i]

    tcols = np.empty((nstep, 3), dtype=np.float32)
    tcols[:, 0] = np.arange(nstep, dtype=np.float32)
    tcols[:, 1] = tcols[:, 0] + 0.5
    tcols[:, 2] = tcols[:, 0] + 1.0
    bias1 = (
        b1[None, None, :] + tcols[:, :, None] * W1[0][None, None, :]
    ).astype(np.float32)
    bias1 = bias1.reshape(nstep * 3, MLP_H).T.copy()

    w1rep = np.repeat(np.asarray(W1[1:], np.float32), C_IN, axis=0)  # [512, 128]
    w1a = np.concatenate(
        [0.5 * w1rep[j * MLP_H : (j + 1) * MLP_H] for j in range(4)], axis=1
    )
    w1b = np.concatenate(
        [0.25 * w1rep[j * MLP_H : (j + 1) * MLP_H] for j in range(4)], axis=1
    )
    w1c = np.concatenate(
        [(1.0 / 6.0) * w1rep[j * MLP_H : (j + 1) * MLP_H] for j in range(4)], axis=1
    )
    sfull = np.repeat(np.eye(HID, dtype=np.float32), C_IN, axis=0)  # [512, 64]
    s6 = np.concatenate(
        [(1.0 / 6.0) * sfull[j * MLP_H : (j + 1) * MLP_H] for j in range(4)], axis=1
    )

    with_b2 = bool(np.any(np.asarray(b2)))
    shared = {
        "bias1": bias1,
        "w1z": np.ascontiguousarray(W1[1:], dtype=np.float16),
        "w1a": np.ascontiguousarray(w1a, dtype=np.float16),
        "w1b": np.ascontiguousarray(w1b, dtype=np.float16),
        "w1c": np.ascontiguousarray(w1c, dtype=np.float16),
        "s6": np.ascontiguousarray(s6, dtype=np.float16),
        "w2": np.ascontiguousarray(W2, dtype=np.float16),
    }
    if with_b2:
        shared["b2c"] = np.ascontiguousarray(
            np.asarray(b2, np.float32).reshape(4, MLP_H), dtype=np.float16
        )
        shared["jsel"] = np.ascontiguousarray(
            np.kron(np.eye(4, dtype=np.float32), np.ones((1, BL // NH), np.float32)),
            dtype=np.float16,
        )

    in_maps = []
    for core in range(NCORES):
        sl = slice(core * BL, (core + 1) * BL)
        m = dict(shared)
        gc = g[sl]  # [BL, nstep, 3, 8]
        arr = gc.transpose(3, 1, 2, 0)  # [8, nstep, 3, BL]
        rep = np.tile(arr, (MLP_H // C_IN, 1, 1, 1))  # [128, nstep, 3, BL]
        m["grep"] = np.ascontiguousarray(
            rep.reshape(MLP_H, nstep * 3 * BL), dtype=np.float16
        )
        z0t = np.ascontiguousarray(z0[sl].T)
        m["z0t"] = z0t
        m["z0h"] = np.ascontiguousarray(z0t, dtype=np.float16)
        in_maps.append(m)
    return in_maps, with_b2


def kernel(coeffs, Wi1, bi1, Wi2, bi2, W1, b1, W2, b2, _nstep: int = NSTEP,
           _trace: bool = False):
    import time as _time
    import sys

    nstep = _nstep
    in_maps, with_b2 = _host_prep(
        coeffs, Wi1, bi1, Wi2, bi2, W1, b1, W2, b2, nstep
    )
    nc = _get_nc(nstep, with_b2)
    t0 = _time.time()
    res = run_bass_kernel_spmd(nc, in_maps, list(range(NCORES)), trace=_trace)
    print(f"[kernel] spmd run (compile+exec): {_time.time()-t0:.1f}s", file=sys.stderr)
    out = np.empty((B, nstep + 1, HID), dtype=np.float32)
    for core in range(NCORES):
        zs = res.results[core]["zs"].reshape(HID, nstep + 1, BL)
        out[core * BL : (core + 1) * BL] = zs.transpose(2, 1, 0)
    if _trace:
        kernel.last_results = res
    return out
